# revision 52
# baseline (speedup 1.0000x reference)
"""Guided channel-wise 3x3 conv (per-pixel weights) on 8 Trainium2 cores.

out[b,c,h,w] = sum_{dh,dw in {-1,0,1}} input[b,c,h+dh,w+dw] * weights[b,c,k(dh,dw),h,w]
with SAME zero padding.  Shapes: input (8,64,128,128) f32,
weights (8,64,9,128,128) f32 -> out (8,64,128,128) f32.

Sharding: pure data parallelism, one batch sample per NeuronCore (B=8 cores).

Strategy (measured path: 81.5us all-fp16 HWDGE baseline -> ~72.8us):
 - Weights are stored int8 in DRAM (per-PARTITION symmetric scale s_p =
   max|w_p|; the s_p/127 factor is folded into the fp16 input windows
   host-side, so the device kernel needs no rescale anywhere).  Measured
   end-to-end rel err ~1.0e-2 < 2e-2 gate (the int8 -> fp16 conversion is
   exact; the error is pure weight quantization).
 - Weight loads go through the gpsimd software-DGE queue as CAST DMAs
   (int8 DRAM -> fp16 SBUF): HBM src bytes halved, and the queue sustains
   ~420 B/ns dst-side across the 16 DMA engines with 16KB descriptors.
 - ALL bulk traffic rides that ONE queue: running the hw + sw queues
   concurrently was measured to COLLAPSE combined throughput (150+150 vs
   420 B/ns for either alone).  Only the tiny ident + first input piece use
   the sync-engine hw queue, which boots ~1us earlier (pre-stream window).
 - Load boundaries are chosen so every steady-state cast DMA is EXACTLY
   8192 elems = one 16KB dst descriptor per partition (sub-16KB descriptors
   measurably starve the engines); the first/last loads are small so the
   first product starts early and only a tiny product trails the last load.
 - DVE does the per-tap products in-place in the fp16 weight buffer
   (0.573 ns/elem windowed, measured) -> ~42.5us, overlapped; the run is
   DMA-stream-bound, not DVE-bound.
 - PE reduces the 9 taps as identity-matmuls accumulating in PSUM (f32),
   ACT drains PSUM -> fp16 out buffer; batched stores are act-gated and
   issued at the stream tail so they never preempt the weight loads.

Per-core layout: 128 SBUF partitions = (half, c) with p = half*64 + c; each
partition holds one 64-row half of one channel plane.  The input is pre-padded
on the host into the per-partition SBUF layout (66 x 130 fp16, zero border).

DMA completion on a DGE queue is NOT in-order across the 16 engines, so every
consumer waits on a semaphore only its own producer DMA increments.  Weight
DRAM layout is (partition, chunk, tap, row, col).
"""

import numpy as np

from concourse import bass, mybir
from concourse.bass_utils import run_bass_kernel_spmd

B, CI, H, W = 8, 64, 128, 128
K = 9
HH = H // 2  # rows per half-plane (64)
PR = HH + 2  # padded rows per partition (66)
PC = W + 2  # padded cols (130)
NP = 128  # SBUF partitions
FP = HH * W  # free elems per partition of one output half-plane (8192)

C = 8  # row-chunks per half-plane
CR = HH // C  # rows per chunk (8)
CH = CR * W  # elems per chunk per partition (1024)
BLK = 512  # matmul moving-dim block (= one PSUM bank of f32)
NB = CH // BLK  # matmul blocks per chunk (2)
NPS = 4  # PSUM chunk buffers (4 x 2 banks = all 8)

WSZ = C * K * CH  # weight elems per partition (73728)

# Weight cast loads: element boundaries chosen so every steady-state DMA is
# EXACTLY 8192 elems = one 16KB dst descriptor per partition (small
# sub-descriptors measurably starve the DMA engines).  The first load is
# small (early first product); the last three are small and tap-aligned so
# only a tiny product remains serial behind the final load.
LOAD_CUTS = (
    [0, 3072, 6144, 9216]
    + [9216 + 8192 * j for j in range(1, 8)]
    + [70656, 72704, 73728]
)
NLOADS = len(LOAD_CUTS) - 1  # 13
# Product groups per chunk: (klo, khi) tap ranges within one dh row.
FULL_PROD = [(0, 3), (3, 6), (6, 9)]
LAST_PROD = [(0, 3), (3, 6), (6, 8), (8, 9)]
PRODS = [FULL_PROD] * (C - 1) + [LAST_PROD]


def loads_covering(lo, hi):
    """Indices of weight loads overlapping elem range [lo, hi)."""
    return [
        j
        for j in range(NLOADS)
        if LOAD_CUTS[j] < hi and LOAD_CUTS[j + 1] > lo
    ]

F16 = mybir.dt.float16
F32 = mybir.dt.float32
I8 = mybir.dt.int8

# input pieces (padded-row boundaries): [0,18) covers chunks 0-1, [18,34)
# covers chunks 2-3, [34,66) covers chunks 4-7
IN_CUT1 = 18 * PC
IN_CUT2 = 34 * PC


def build_bass():
    nc = bass.Bass()
    ident_d = nc.declare_dram_parameter("ident", [NP, NP], F16, isOutput=False)
    inp_d = nc.declare_dram_parameter("input", [NP, PR * PC], F16, isOutput=False)
    wts_d = nc.declare_dram_parameter("weights", [NP, WSZ], I8, isOutput=False)
    out_d = nc.declare_dram_parameter("out", [NP, FP], F16, isOutput=True)

    from contextlib import ExitStack

    with ExitStack() as ctx:
        ident = ctx.enter_context(nc.sbuf_tensor("ident_s", [NP, NP], F16))
        in_pad = ctx.enter_context(nc.sbuf_tensor("in_pad", [NP, PR * PC], F16))
        wt = ctx.enter_context(nc.sbuf_tensor("wt", [NP, WSZ], F16))
        out_t = ctx.enter_context(nc.sbuf_tensor("out_t", [NP, FP], F16))
        ps = [
            ctx.enter_context(nc.psum_tensor(f"ps{j}", [NP, CH], F32))
            for j in range(NPS)
        ]
        block = ctx.enter_context(nc.Block())
        isem = ctx.enter_context(nc.semaphore("isem"))
        nsem = [ctx.enter_context(nc.semaphore(f"nsem{i}")) for i in range(3)]
        lsem = [
            ctx.enter_context(nc.semaphore(f"lsem{j}")) for j in range(NLOADS)
        ]
        dve_sem = ctx.enter_context(nc.semaphore("dve_sem"))
        pe_sem = ctx.enter_context(nc.semaphore("pe_sem"))
        act_sem = ctx.enter_context(nc.semaphore("act_sem"))
        st_sem = ctx.enter_context(nc.semaphore("st_sem"))
        st2_sem = ctx.enter_context(nc.semaphore("st2_sem"))

        # weight region for (chunk c, tap k): contiguous CH elems
        def woff(c, k):
            return c * (K * CH) + k * CH

        def custom_ap(base, pattern, offset):
            a = base.copy()
            a.ap[:] = pattern
            a.offset = offset
            return a

        N_ST = 3  # batched stores: chunks 0-3, 4-6, then the last chunk

        @block.sync
        def _(sync):
            # The tiny ident + first input piece ride the hw queue, which
            # boots ~1us before the gpsimd sw queue -- they mostly complete
            # in the pre-stream window, and their small descriptors stay out
            # of the sw weight stream.
            sync.dma_start(out=ident[:], in_=ident_d[:]).then_inc(isem, 16)
            sync.dma_start(out=in_pad[:, :IN_CUT1], in_=inp_d[:, :IN_CUT1]).then_inc(
                nsem[0], 16
            )
            # The LAST store rides the hw queue (sw queue idle by then, no
            # collapse risk): if gpsimd issued it, gpsimd's ~3us engine
            # DRAIN would only start after act9 (~69us) and serialize into
            # the kernel tail; this way that drain overlaps the compute.
            sync.wait_ge(act_sem, C + NB - 1)
            sync.dma_start(out=out_d[:, 7 * CH :], in_=out_t[:, 7 * CH :]).then_inc(
                st2_sem, 16
            )
            sync.wait_ge(st_sem, 32)
            sync.wait_ge(st2_sem, 16)

        @block.gpsimd
        def _(gpsimd):
            # The BULK rides the gpsimd software-DGE queue as one ordered
            # stream: running the hw and sw queues concurrently for bulk was
            # measured to COLLAPSE combined throughput (150+150 vs 420 B/ns
            # alone).  Order: input pieces at need-time, 16KB-aligned weight
            # casts, act-gated stores at the tail.
            for j in range(NLOADS):
                lo, hi = LOAD_CUTS[j], LOAD_CUTS[j + 1]
                gpsimd.dma_start(out=wt[:, lo:hi], in_=wts_d[:, lo:hi]).then_inc(
                    lsem[j], 16
                )
                if j == 2:
                    gpsimd.dma_start(
                        out=in_pad[:, IN_CUT1:IN_CUT2], in_=inp_d[:, IN_CUT1:IN_CUT2]
                    ).then_inc(nsem[1], 16)
                if j == 4:
                    gpsimd.dma_start(
                        out=in_pad[:, IN_CUT2:], in_=inp_d[:, IN_CUT2:]
                    ).then_inc(nsem[2], 16)
            # Batched stores, issued AFTER all loads on the same queue so
            # they can never preempt the weight stream.  Gated late (act6)
            # so idle engines cannot pull store descriptors while the tail
            # weight loads are still draining.
            gpsimd.wait_ge(act_sem, 6)
            gpsimd.dma_start(out=out_d[:, : 4 * CH], in_=out_t[:, : 4 * CH]).then_inc(
                st_sem, 16
            )
            gpsimd.wait_ge(act_sem, C - 1)
            gpsimd.dma_start(
                out=out_d[:, 4 * CH : 7 * CH], in_=out_t[:, 4 * CH : 7 * CH]
            ).then_inc(st_sem, 16)

        IN_WAIT = {0: 0, 2: 1, 4: 2}  # input piece needed before chunk c

        @block.vector
        def _(vector):
            # products, in-place into the cast weight regions; each product
            # group is ONE fused tensor_tensor: the group's taps become a
            # third free dim (weight regions stride CH apart; the matching
            # input windows stride 1 apart in dw)
            max_load = -1
            for c in range(C):
                if c in IN_WAIT:
                    vector.wait_ge(nsem[IN_WAIT[c]], 16)
                r0 = c * CR
                for klo, khi in PRODS[c]:
                    nt = khi - klo
                    dh, dw = klo // 3, klo % 3
                    # weight loads are consumed strictly in order, so only
                    # newly-covered load indices need a wait
                    for j in loads_covering(woff(c, klo), woff(c, khi)):
                        if j > max_load:
                            vector.wait_ge(lsem[j], 16)
                            max_load = j
                    wv = custom_ap(
                        wt[:],
                        [[WSZ, NP], [CH, nt], [W, CR], [1, W]],
                        woff(c, klo),
                    )
                    iv = custom_ap(
                        in_pad[:],
                        [[PR * PC, NP], [1, nt], [PC, CR], [1, W]],
                        (dh + r0) * PC + dw,
                    )
                    vector.tensor_tensor(
                        out=wv, in0=wv, in1=iv, op=mybir.AluOpType.mult
                    ).then_inc(dve_sem, 1)

        # dve_sem value after the product covering tap k of chunk c
        gbase = [sum(len(PRODS[cc]) for cc in range(c)) for c in range(C)]

        def dve_count(c, k):
            for g, (klo, khi) in enumerate(PRODS[c]):
                if klo <= k < khi:
                    return gbase[c] + g + 1
            raise AssertionError

        @block.tensor
        def _(tensor):
            # 9-tap reduction: psum[chunk] += I @ p_k (f32 accumulation)
            tensor.wait_ge(isem, 16)
            last_wait = 0
            for c in range(C):
                if c >= NPS:
                    tensor.wait_ge(act_sem, c - NPS + 1)
                pb = ps[c % NPS]
                for k in range(K):
                    for b in range(NB):
                        if dve_count(c, k) > last_wait:
                            last_wait = dve_count(c, k)
                            tensor.wait_ge(dve_sem, last_wait)
                        inst = tensor.matmul(
                            out=pb[:, b * BLK : (b + 1) * BLK],
                            lhsT=ident[:],
                            rhs=wt[:, woff(c, k) + b * BLK : woff(c, k) + (b + 1) * BLK],
                            start=(k == 0),
                            stop=(k == K - 1),
                            skip_group_check=True,
                        )
                        if k == K - 1 and (c == C - 1 or b == NB - 1):
                            # last chunk: per-bank completion for a finer tail
                            inst.then_inc(pe_sem, 1)

        @block.scalar
        def _(scalar):
            # drain PSUM -> fp16 out buffer
            for c in range(C - 1):
                scalar.wait_ge(pe_sem, c + 1)
                scalar.activation(
                    out=out_t[:, c * CH : (c + 1) * CH],
                    in_=ps[c % NPS][:],
                    func=mybir.ActivationFunctionType.Copy,
                ).then_inc(act_sem, 1)
            # last chunk: per-bank copy for a finer tail
            c = C - 1
            for b in range(NB):
                lo = c * CH + b * BLK
                scalar.wait_ge(pe_sem, c + b + 1)
                scalar.activation(
                    out=out_t[:, lo : lo + BLK],
                    in_=ps[c % NPS][:, b * BLK : (b + 1) * BLK],
                    func=mybir.ActivationFunctionType.Copy,
                ).then_inc(act_sem, 1)

    return nc


def _prep_weights(w):
    """(64,9,128,128) f32 -> int8 [128, C*K*CH] + per-partition scales [128].

    partition p = half*64 + channel; free = (row-chunk, tap, row-in-chunk, col)
    so each (chunk, tap-range) is one contiguous cast DMA per partition.
    """
    wr = (
        w.reshape(CI, K, 2, C, CR, W)
        .transpose(2, 0, 3, 1, 4, 5)
        .reshape(NP, WSZ)
        .astype(np.float32)
    )
    s = np.abs(wr).max(axis=1)  # [128]
    s = np.maximum(s, 1e-30)
    wi8 = np.rint(wr * (127.0 / s[:, None])).astype(np.int8)
    return np.ascontiguousarray(wi8), s


def _prep_input(x, s):
    """(64,128,128) f32 -> (128, 66*130) fp16 padded layout, scaled by s_p/127."""
    pad = np.zeros((CI, H + 2, W + 2), dtype=np.float32)
    pad[:, 1 : H + 1, 1 : W + 1] = x
    win = np.stack([pad[:, 0:PR, :], pad[:, HH : HH + PR, :]], axis=0)
    win = win.reshape(NP, PR * PC) * (s[:, None] / 127.0)
    return np.ascontiguousarray(win.astype(np.float16))


def _unprep_out(o):
    """(128, 64*128) fp16 -> (64,128,128) f32."""
    return np.ascontiguousarray(
        np.asarray(o)
        .astype(np.float32)
        .reshape(2, CI, HH, W)
        .transpose(1, 0, 2, 3)
        .reshape(CI, H, W)
    )


_IDENT = np.eye(NP, dtype=np.float16)

_NC = None


def _get_nc():
    global _NC
    if _NC is None:
        _NC = build_bass()
    return _NC


def make_in_maps(input, weights):
    input = np.asarray(input, dtype=np.float32)
    weights = np.asarray(weights, dtype=np.float32)
    maps = []
    for b in range(B):
        wi8, s = _prep_weights(weights[b])
        maps.append(
            {
                "ident": _IDENT,
                "input": _prep_input(input[b], s),
                "weights": wi8,
            }
        )
    return maps


def kernel(input, weights):
    nc = _get_nc()
    in_maps = make_in_maps(input, weights)
    res = run_bass_kernel_spmd(nc, in_maps, list(range(B)))
    return np.stack([_unprep_out(res.results[b]["out"]) for b in range(B)], axis=0)
